# revision 1
# baseline (speedup 1.0000x reference)
"""Trainium2 Bass kernel for nn_GCLMemory (content-addressed memory read weights).

Full computation per batch sample b:
    dots[n]  = <keys[b,n,:], k[b,:]>
    cos[n]   = dots[n] / (max(||keys[b,n]||,eps) * max(||k[b]||,eps))
    wc       = softmax(beta[b] * cos)
    top-32 mask (1.0 at top-32 of wc, 1e-16 elsewhere), renormalize,
    w        = wc**gamma[b], renormalize.

Sharding: data-parallel over batch. 8 cores x 16 samples. Host pre-transposes
each core's keys slice to [2, 128, 16, 2048] (K on partitions) so the PE can
contract over K. Per (sample b, k-chunk q) the kernel streams a [128, 2048]
tile, squares it on ACT, and issues matmuls with lhsT = kvec_b (dots) and
lhsT = ones (row sumsq), accumulating into PSUM rows at partition offset b,
which yields row-major [16, 512] dots/sumsq tiles with no transposes.

Tail identity used: the intermediate renormalizations cancel, so
    w = em**gamma / sum(em**gamma),  em = e*1 at top-32, e*1e-16 elsewhere,
    e = exp(beta*cos)   (no max-subtraction needed: |beta*cos| <= ~5.5).
Top-32 found with 4 rounds of DVE max8 + match_replace(0.0) on a copy of e;
the zeroed copy provides the mask via  em = (e - e_rem) + 1e-16*e_rem.
"""

import sys

import numpy as np

sys.path.insert(0, "/opt/trn_rl_repo")

import concourse.bass as bass
import concourse.mybir as mybir
from concourse.bass_utils import run_bass_kernel_spmd
from concourse.tile import TileContext
from concourse import masks

F32 = mybir.dt.float32
Alu = mybir.AluOpType
Act = mybir.ActivationFunctionType

# ---------------------------------------------------------------------------
# This container's walrus build only accepts a single sem-wait command per
# instruction ("Too many sync wait commands" in CoreV3GenImpl otherwise), but
# Tile's exit drain aggregates one wait per busy processor. Split any
# multi-wait instruction into preceding single-wait Drains on the same engine.
# ---------------------------------------------------------------------------
_WAIT_LIMIT = 1


def _split_multi_waits(bir_bytes: bytes, limit: int = _WAIT_LIMIT) -> bytes:
    import orjson
    d = orjson.loads(bir_bytes)
    n_split = 0
    for fn in d.get("functions", []):
        for bb in fn.get("blocks", []):
            out = []
            for inst in bb.get("instructions", []):
                si = inst.get("sync_info") or {}
                waits = si.get("on_wait") or []
                if len(waits) > limit:
                    n_split += 1
                    chunks = [waits[i:i + limit]
                              for i in range(0, len(waits), limit)]
                    for j, ch in enumerate(chunks[:-1]):
                        carrier = {
                            "engine": inst["engine"],
                            "ins": [],
                            "is_reset_sema": False,
                            "name": f"{inst['name']}__w{j}",
                            "opcode": "Drain",
                            "outs": [],
                            "sync_info": {"on_update": [], "on_wait": ch},
                        }
                        if "debug" in inst:
                            carrier["debug"] = inst["debug"]
                        out.append(carrier)
                    si["on_wait"] = chunks[-1]
                out.append(inst)
            bb["instructions"] = out
    return orjson.dumps(d)


def _install_wait_split_hook():
    from concourse import bass2jax
    orig = bass2jax.compile_bir_kernel
    if getattr(orig, "_wait_split_wrapped", False):
        return

    def wrapped(bir_bytes, *args, **kwargs):
        return orig(_split_multi_waits(bir_bytes), *args, **kwargs)

    wrapped._wait_split_wrapped = True
    bass2jax.compile_bir_kernel = wrapped


_install_wait_split_hook()

B, N, K = 128, 2048, 256
M = 8            # cores
BPC = B // M     # samples per core
KQ = K // 128    # contraction chunks
NT = N // 512    # psum column tiles
CAND = 32
EPS = 1e-8


def build_nc(n_reps: int = 1, tail: bool = True):
    nc = bass.Bass()
    keysT = nc.declare_dram_parameter("keysT", [KQ, 128, BPC, N], F32, isOutput=False)
    kvT17 = nc.declare_dram_parameter("kvT17", [KQ, 128, 32], F32, isOutput=False)
    kvr = nc.declare_dram_parameter("kvr", [BPC, K], F32, isOutput=False)
    beta = nc.declare_dram_parameter("beta", [BPC, 1], F32, isOutput=False)
    gamma = nc.declare_dram_parameter("gamma", [BPC, 1], F32, isOutput=False)
    out = nc.declare_dram_parameter("out", [BPC, N], F32, isOutput=True)

    with TileContext(nc) as tc:
        with (
            tc.tile_pool(name="const", bufs=1) as cpool,
            tc.tile_pool(name="stream", bufs=3) as spool,
            tc.tile_pool(name="psum", bufs=1, space="PSUM") as ppool,
        ):
          for _rep in range(n_reps):
              ones = cpool.tile([128, 1], F32, tag="ones")
              nc.vector.memset(ones[:], 1.0)
              kv = []
              for q in range(KQ):
                  t = cpool.tile([128, 32], F32, name=f"kv{q}", tag=f"kv{q}")
                  nc.sync.dma_start(out=t[:], in_=kvT17[q])
                  kv.append(t)
              kvr_t = cpool.tile([BPC, K], F32, tag="kvr")
              nc.sync.dma_start(out=kvr_t[:], in_=kvr[:])
              beta_t = cpool.tile([BPC, 1], F32, tag="beta")
              nc.sync.dma_start(out=beta_t[:], in_=beta[:])
              gamma_t = cpool.tile([BPC, 1], F32, tag="gamma")
              nc.sync.dma_start(out=gamma_t[:], in_=gamma[:])

              # lnscale = ln(beta) - 0.5*ln(||k||^2): avoids sqrt+reciprocal
              # (InstReciprocal is ~13us on this platform).
              qsq = cpool.tile([BPC, K], F32, tag="qsq")
              qn2 = cpool.tile([BPC, 1], F32, tag="qn2")
              nc.scalar.activation(qsq[:], kvr_t[:], Act.Square,
                                   accum_out=qn2[:])
              lnb = cpool.tile([BPC, 1], F32, tag="lnb")
              nc.scalar.activation(lnb[:], beta_t[:], Act.Ln)
              lnq2 = cpool.tile([BPC, 1], F32, tag="lnq2")
              nc.scalar.activation(lnq2[:], qn2[:], Act.Ln)
              lnscale = cpool.tile([BPC, 1], F32, tag="lnscale")
              nc.vector.scalar_tensor_tensor(
                  lnscale[:], lnq2[:], -0.5, lnb[:],
                  op0=Alu.mult, op1=Alu.add)

              D = cpool.tile([BPC, N], F32, tag="D")
              S = cpool.tile([BPC, N], F32, tag="S")
              # Row-orientation stream: stationary lhsT = [kv_0..kv_15 | ones]
              # [128, 17]; rhs = 512-col slices of raw (dots) or squared
              # (sumsq) stream tiles. Each matmul's useful output row is b
              # (dots, raw stream) or 16 (sumsq, sq stream); jobs are packed
              # 3-per-PSUM-tile at base partitions 0/32/64 (the only legal
              # bases), staged to SBUF with one [81,512] copy, and the useful
              # rows land in D/S via tiny SBUF->SBUF row DMAs.
              GRP = 2  # samples per stream tile: 16KB DMA lines
              jobs = []
              for g in range(BPC // GRP):
                  for j in range(GRP):
                      for kind in range(2):
                          jobs.append((g, j, kind))
              stream_tiles = {}
              cur = None
              cur_jobs = []
              slot = 3
              tile_i = 0

              def flush_tile():
                  nrows = 32 * len(cur_jobs)
                  stag = spool.tile([96, N], F32, name=f"stag{tile_i}",
                                    tag="stag", bufs=2)
                  nc.vector.tensor_copy(stag[0:nrows, :], cur[0:nrows, :])
                  for (s_, b_, kind_) in cur_jobs:
                      row = 32 * s_ + (b_ if kind_ == 0 else 16)
                      dst = D if kind_ == 0 else S
                      nc.sync.dma_start(out=dst[b_:b_ + 1, :],
                                        in_=stag[row:row + 1, :])

              for (g, j, kind) in jobs:
                  if g not in stream_tiles:
                      raws, sqs = [], []
                      for q in range(KQ):
                          raw = spool.tile([128, GRP * N], F32,
                                           name=f"raw{q}", tag=f"raw{q}",
                                           bufs=2)
                          nc.sync.dma_start(
                              out=raw[:],
                              in_=keysT[q, :, GRP * g:GRP * (g + 1), :])
                          sq = spool.tile([128, GRP * N], F32,
                                          name=f"sq{q}", tag=f"sq{q}", bufs=2)
                          nc.scalar.square(sq[:], raw[:])
                          raws.append(raw)
                          sqs.append(sq)
                      stream_tiles[g] = (raws, sqs)
                  raws, sqs = stream_tiles[g]
                  if slot == 3:
                      if cur is not None:
                          flush_tile()
                      tile_i += 1
                      cur = ppool.tile([96, N], F32, name=f"P{tile_i}",
                                       tag="P", bufs=2)
                      cur_jobs = []
                      slot = 0
                  src = raws if kind == 0 else sqs
                  for t in range(N // 512):
                      csl = slice(j * N + 512 * t, j * N + 512 * (t + 1))
                      for q in range(KQ):
                          nc.tensor.matmul(
                              cur[32 * slot:32 * slot + 32,
                                  512 * t:512 * (t + 1)],
                              kv[q][:, 0:32], src[q][:, csl],
                              start=(q == 0), stop=(q == KQ - 1))
                  cur_jobs.append((slot, GRP * g + j, kind))
                  slot += 1
              if cur is not None:
                  flush_tile()

              # ---- tail, all on [BPC, N] rows ----
              # rfold = exp(-0.5*ln(S) + lnscale) = beta/(||k||*||keys_n||)
              lnS = cpool.tile([BPC, N], F32, tag="t1", name="lnS")
              nc.scalar.activation(lnS[:], S[:], Act.Ln)
              rfold = cpool.tile([BPC, N], F32, tag="t2", name="rfold")
              nc.scalar.activation(rfold[:], lnS[:], Act.Exp, scale=-0.5,
                                   bias=lnscale[:])
              logits = cpool.tile([BPC, N], F32, tag="t1", name="logits")
              nc.vector.tensor_tensor(logits[:], D[:], rfold[:], Alu.mult)
              e1 = cpool.tile([BPC, N], F32, tag="t2", name="e1")
              nc.scalar.activation(e1[:], logits[:], Act.Exp)
              e2 = cpool.tile([BPC, N], F32, tag="t1", name="e2")
              nc.vector.tensor_copy(e2[:], e1[:])
              m8 = cpool.tile([BPC, 8], F32, tag="m8", name="m8")
              for rd in range(CAND // 8):
                  nc.vector.max(m8[:], e2[:])
                  if rd < CAND // 8 - 1:
                      nc.vector.match_replace(e2[:], m8[:], e2[:], 0.0)
              # em = etop + 1e-16*e1, etop = e1 * (e1 >= t32)
              etop = cpool.tile([BPC, N], F32, tag="t3", name="etop")
              nc.vector.scalar_tensor_tensor(
                  etop[:], e1[:], m8[:, 7:8], e1[:],
                  op0=Alu.is_ge, op1=Alu.mult)
              em = cpool.tile([BPC, N], F32, tag="t1", name="em")
              nc.vector.scalar_tensor_tensor(
                  em[:], e1[:], 1e-16, etop[:], op0=Alu.mult, op1=Alu.add)
              lgm = cpool.tile([BPC, N], F32, tag="t2", name="lgm")
              nc.scalar.activation(lgm[:], em[:], Act.Ln)
              wt = cpool.tile([BPC, N], F32, tag="t3", name="wt")
              zsum = cpool.tile([BPC, 1], F32, tag="zsum")
              nc.scalar.activation(wt[:], lgm[:], Act.Exp, scale=gamma_t[:],
                                   accum_out=zsum[:])
              zr = cpool.tile([BPC, 1], F32, tag="zr")
              nc.vector.reciprocal(zr[:], zsum[:])
              w = cpool.tile([BPC, N], F32, tag="t1", name="w")
              nc.vector.tensor_scalar(w[:], wt[:], zr[:], None, Alu.mult)
              nc.sync.dma_start(out=out[:], in_=w[:])
    return nc


def shard_inputs(k, beta, gamma, keys):
    k = np.ascontiguousarray(k, dtype=np.float32)
    beta = np.ascontiguousarray(beta, dtype=np.float32)
    gamma = np.ascontiguousarray(gamma, dtype=np.float32)
    keys = np.ascontiguousarray(keys, dtype=np.float32)
    in_maps = []
    for c in range(M):
        sl = slice(c * BPC, (c + 1) * BPC)
        kc = np.ascontiguousarray(k[sl])                       # [BPC, K]
        keysTc = np.ascontiguousarray(
            keys[sl].transpose(2, 0, 1)).reshape(KQ, 128, BPC, N)
        kvTc = np.ascontiguousarray(kc.T).reshape(KQ, 128, BPC)
        kvT17c = np.zeros((KQ, 128, 32), np.float32)
        kvT17c[:, :, 0:BPC] = kvTc
        kvT17c[:, :, 16] = 1.0
        in_maps.append({
            "keysT": keysTc,
            "kvT17": kvT17c,
            "kvr": kc,
            "beta": np.ascontiguousarray(beta[sl]),
            "gamma": np.ascontiguousarray(gamma[sl]),
        })
    return in_maps


_NC_CACHE = None


def kernel(k=None, beta=None, gamma=None, keys=None, candidates=None, **_ignored):
    assert int(candidates) == CAND, f"kernel hardcoded for candidates=32, got {candidates}"
    assert keys.shape == (B, N, K), keys.shape
    global _NC_CACHE
    if _NC_CACHE is None:
        _NC_CACHE = build_nc()
    in_maps = shard_inputs(k, beta, gamma, keys)
    res = run_bass_kernel_spmd(_NC_CACHE, in_maps, list(range(M))).results
    return np.concatenate([res[c]["out"] for c in range(M)], axis=0)



# revision 5
# speedup vs baseline: 1.6428x; 1.6428x over previous
"""Trainium2 Bass kernel for nn_GCLMemory (content-addressed memory read weights).

Full computation per batch sample b:
    dots[n]  = <keys[b,n,:], k[b,:]>
    cos[n]   = dots[n] / (max(||keys[b,n]||,eps) * max(||k[b]||,eps))
    wc       = softmax(beta[b] * cos)
    top-32 mask, renormalize, w = wc**gamma[b], renormalize.

Tail identity: the intermediate renormalizations cancel, so
    w = exp(gamma*logits) / sum  over the top-32 logits positions,
    logits = beta*cos.  Non-top leakage (1e-16 factor) is below fp32
    noise after **gamma, so it is dropped.

Sharding: data-parallel over batch. 8 cores x 16 samples.

Device-side layout (per core):
  - Host pre-transposes + casts keys to bf16: keysT [KQ=2, 128, 16, 2048]
    (K on partitions).  bf16 halves HBM traffic and runs the PE at
    1 cycle/row instead of fp32's 4.
  - lhsT kvT [KQ, 128, 32] bf16: col s = kvec of sample s, col 16 = ones.
    Per (sample, kind) job the PE streams 512-col slices of the raw
    (dots) or squared (sumsq) stream tile; useful output rows are s
    (dots) and 16 (sumsq).  4 jobs pack one [128, 2048] PSUM tile at
    partition bases 0/32/64/96.
  - PSUM tiles are bulk-copied to SBUF on ACT; per-job useful rows are
    extracted by DMA into D/S in a [128, 256] layout with partition
    p = 8*s + c (c = 256-col chunk of n), so the whole tail runs with
    128 active partitions (DVE/ACT cost scales with free size only).
  - Tail: rsb = exp(-0.5*ln(S) + ln(beta/||k||)); lg30 = D*rsb + 30
    (the +30 shift keeps all values positive for the match_replace
    top-k trick and is undone by the Exp bias); per-partition top-32
    via 4x max8 + match_replace; cross-chunk merge after a [128,32] ->
    [16,256] DMA reshape; threshold mask via is_ge; w = Exp(gamma*x -
    30*gamma) with accumulate; renorm via DMA-reshaped partial sums.
"""

import sys

import numpy as np

sys.path.insert(0, "/opt/trn_rl_repo")

import concourse.bass as bass
import concourse.mybir as mybir
from concourse.bass_utils import run_bass_kernel_spmd
from concourse.tile import TileContext

F32 = mybir.dt.float32
F32R = mybir.dt.float32r
BF16 = mybir.dt.bfloat16
Alu = mybir.AluOpType
Act = mybir.ActivationFunctionType
AxL = mybir.AxisListType

# ---------------------------------------------------------------------------
# This container's walrus build only accepts a single sem-wait command per
# instruction ("Too many sync wait commands" in CoreV3GenImpl otherwise), but
# Tile's exit drain aggregates one wait per busy processor. Split any
# multi-wait instruction into preceding single-wait Drains on the same engine.
# ---------------------------------------------------------------------------
_WAIT_LIMIT = 1


def _split_multi_waits(bir_bytes: bytes, limit: int = _WAIT_LIMIT) -> bytes:
    import orjson
    d = orjson.loads(bir_bytes)
    n_split = 0
    for fn in d.get("functions", []):
        for bb in fn.get("blocks", []):
            out = []
            for inst in bb.get("instructions", []):
                si = inst.get("sync_info") or {}
                waits = si.get("on_wait") or []
                if len(waits) > limit:
                    n_split += 1
                    chunks = [waits[i:i + limit]
                              for i in range(0, len(waits), limit)]
                    for j, ch in enumerate(chunks[:-1]):
                        carrier = {
                            "engine": inst["engine"],
                            "ins": [],
                            "is_reset_sema": False,
                            "name": f"{inst['name']}__w{j}",
                            "opcode": "Drain",
                            "outs": [],
                            "sync_info": {"on_update": [], "on_wait": ch},
                        }
                        if "debug" in inst:
                            carrier["debug"] = inst["debug"]
                        out.append(carrier)
                    si["on_wait"] = chunks[-1]
                out.append(inst)
            bb["instructions"] = out
    return orjson.dumps(d)


def _install_wait_split_hook():
    from concourse import bass2jax
    orig = bass2jax.compile_bir_kernel
    if getattr(orig, "_wait_split_wrapped", False):
        return

    def wrapped(bir_bytes, *args, **kwargs):
        return orig(_split_multi_waits(bir_bytes), *args, **kwargs)

    wrapped._wait_split_wrapped = True
    bass2jax.compile_bir_kernel = wrapped


_install_wait_split_hook()

B, N, K = 128, 2048, 256
M = 8            # cores
BPC = B // M     # samples per core
KQ = K // 128    # contraction chunks
NT = N // 512    # psum column tiles per sample
CAND = 32
GRP = 2          # samples per stream tile (8KB bf16 DMA lines)
NG = BPC // GRP  # stream groups
EPS = 1e-8
SHIFT = 30.0     # logits shift: keeps lg30 > 0 for the match_replace trick


def build_nc():
    nc = bass.Bass()
    keysT = nc.declare_dram_parameter("keysT", [KQ, 128, BPC, N], F32R,
                                      isOutput=False)
    kvTd = nc.declare_dram_parameter("kvTd", [128, BPC * KQ * 32], F32R,
                                     isOutput=False)
    onesT = nc.declare_dram_parameter("onesT", [128, BPC * 32], F32R,
                                      isOutput=False)
    lnsb = nc.declare_dram_parameter("lnsb", [128, 1], F32, isOutput=False)
    gam = nc.declare_dram_parameter("gam", [128, 1], F32, isOutput=False)
    ng30g = nc.declare_dram_parameter("ng30g", [128, 1], F32, isOutput=False)
    out = nc.declare_dram_parameter("out", [128, N // 8], F32, isOutput=True)

    with TileContext(nc) as tc:
        with (
            tc.tile_pool(name="const", bufs=1) as cpool,
            tc.tile_pool(name="stream", bufs=3) as spool,
            tc.tile_pool(name="psum", bufs=1, space="PSUM") as ppool,
        ):
            # lhsT banks: per (sample s, q) a [128,32] fp32r tile with
            # kvec_s chunk q at col s (zeros elsewhere); per sample a ones
            # tile with 1.0 at col 16+s.  fp32r matmuls must write PSUM
            # partition base 0, so all 32 jobs accumulate into ONE shared
            # [32, 2048] PSUM tile: rows 0-15 collect dots (col s -> row s),
            # rows 16-31 collect sumsq; zero lhsT columns contribute 0.
            kvAll = cpool.tile([128, BPC * KQ * 32], F32R, tag="kvAll")
            nc.sync.dma_start(out=kvAll[:], in_=kvTd[:])
            oneAll = cpool.tile([128, BPC * 32], F32R, tag="oneAll")
            nc.sync.dma_start(out=oneAll[:], in_=onesT[:])
            lnsb_t = cpool.tile([128, 1], F32, tag="lnsb")
            nc.sync.dma_start(out=lnsb_t[:], in_=lnsb[:])
            gam_t = cpool.tile([128, 1], F32, tag="gam")
            nc.sync.dma_start(out=gam_t[:], in_=gam[:])
            ng30g_t = cpool.tile([128, 1], F32, tag="ng30g")
            nc.sync.dma_start(out=ng30g_t[:], in_=ng30g[:])

            D = cpool.tile([128, 256], F32, tag="D")
            S = cpool.tile([128, 256], F32, tag="S")

            Pt = ppool.tile([32, N], F32, name="Pt", tag="P", bufs=1)
            for g in range(NG):
                raws, sqs = [], []
                for q in range(KQ):
                    raw = spool.tile([128, GRP * N], F32R, name=f"raw{q}_{g}",
                                     tag=f"raw{q}", bufs=2)
                    eng = nc.sync if q == 0 else nc.scalar
                    eng.dma_start(out=raw[:],
                                  in_=keysT[q, :, GRP * g:GRP * (g + 1), :])
                    sq = spool.tile([128, GRP * N], F32R, name=f"sq{q}_{g}",
                                    tag=f"sq{q}", bufs=2)
                    if q == 0:
                        nc.scalar.square(sq[:], raw[:])
                    else:
                        nc.vector.tensor_tensor(sq[:], raw[:], raw[:],
                                                Alu.mult)
                    raws.append(raw)
                    sqs.append(sq)
                for j in range(GRP):
                    s = GRP * g + j
                    for kind in range(2):
                        for q in range(KQ):
                            if kind == 0:
                                lhsT = kvAll[:, (s * KQ + q) * 32:
                                             (s * KQ + q) * 32 + 32]
                                rhs_t = raws[q]
                            else:
                                lhsT = oneAll[:, s * 32:s * 32 + 32]
                                rhs_t = sqs[q]
                            first = (g == 0 and j == 0 and kind == 0
                                     and q == 0)
                            last = (g == NG - 1 and j == GRP - 1
                                    and kind == 1 and q == KQ - 1)
                            for t_ in range(NT):
                                nc.tensor.matmul(
                                    Pt[0:32, 512 * t_:512 * (t_ + 1)],
                                    lhsT,
                                    rhs_t[:, j * N + 512 * t_:
                                          j * N + 512 * (t_ + 1)],
                                    start=first, stop=last)
            stag = cpool.tile([32, N], F32, tag="stag")
            nc.scalar.copy(stag[:], Pt[:])
            nc.sync.dma_start(out=D[:], in_=stag[0:16, :])
            nc.sync.dma_start(out=S[:], in_=stag[16:32, :])

            # ---- tail on [128, 256]: partition p = 8*sample + chunk ----
            lnS = cpool.tile([128, 256], F32, tag="t1", name="lnS")
            nc.scalar.activation(lnS[:], S[:], Act.Ln)
            # rsb = exp(-0.5*lnS + ln(beta/qn)) = beta/(qn*sqrt(S))
            rsb = cpool.tile([128, 256], F32, tag="t2", name="rsb")
            nc.scalar.activation(rsb[:], lnS[:], Act.Exp, scale=-0.5,
                                 bias=lnsb_t[:])
            lg = cpool.tile([128, 256], F32, tag="t1", name="lg")
            nc.vector.tensor_tensor(lg[:], D[:], rsb[:], Alu.mult)
            lg30 = cpool.tile([128, 256], F32, tag="t3", name="lg30")
            nc.vector.tensor_scalar(lg30[:], lg[:], SHIFT, None, Alu.add)

            work = cpool.tile([128, 256], F32, tag="t2", name="work")
            nc.vector.tensor_copy(work[:], lg30[:])
            cand = cpool.tile([128, 32], F32, tag="cand")
            for r in range(4):
                nc.vector.max(cand[:, 8 * r:8 * r + 8], work[:])
                if r < 3:
                    nc.vector.match_replace(work[:], cand[:, 8 * r:8 * r + 8],
                                            work[:], 0.0)
            # [128, 32] -> [16, 256] flat-order reshape: per-sample merge
            candT = cpool.tile([16, 256], F32, tag="candT")
            nc.sync.dma_start(out=candT[:], in_=cand[:])
            m8f = cpool.tile([16, 8], F32, tag="m8f")
            for r in range(4):
                nc.vector.max(m8f[:], candT[:])
                if r < 3:
                    nc.vector.match_replace(candT[:], m8f[:], candT[:], 0.0)
            # broadcast t32 [16,1] -> [128,1] (p = 8s+c <- s)
            t32r = cpool.tile([16, 8], F32, tag="t32r")
            nc.vector.tensor_copy(t32r[:, 0:1], m8f[:, 7:8])
            nc.vector.tensor_copy(t32r[:, 1:2], t32r[:, 0:1])
            nc.vector.tensor_copy(t32r[:, 2:4], t32r[:, 0:2])
            nc.vector.tensor_copy(t32r[:, 4:8], t32r[:, 0:4])
            t32b = cpool.tile([128, 1], F32, tag="t32b")
            nc.sync.dma_start(out=t32b[:], in_=t32r[:])

            msk = cpool.tile([128, 256], F32, tag="t1", name="msk")
            nc.vector.scalar_tensor_tensor(
                msk[:], lg30[:], t32b[:], lg30[:],
                op0=Alu.is_ge, op1=Alu.mult)
            wt = cpool.tile([128, 256], F32, tag="t2", name="wt")
            zp = cpool.tile([128, 1], F32, tag="zp")
            nc.scalar.activation(wt[:], msk[:], Act.Exp, scale=gam_t[:],
                                 bias=ng30g_t[:], accum_out=zp[:])
            zs = cpool.tile([16, 8], F32, tag="zs")
            nc.sync.dma_start(out=zs[:], in_=zp[:])
            zsum = cpool.tile([16, 1], F32, tag="zsum")
            nc.vector.tensor_reduce(zsum[:], zs[:], axis=AxL.X, op=Alu.add)
            zr = cpool.tile([16, 1], F32, tag="zr")
            nc.vector.reciprocal(zr[:], zsum[:])
            zrr = cpool.tile([16, 8], F32, tag="zrr")
            nc.vector.tensor_copy(zrr[:, 0:1], zr[:])
            nc.vector.tensor_copy(zrr[:, 1:2], zrr[:, 0:1])
            nc.vector.tensor_copy(zrr[:, 2:4], zrr[:, 0:2])
            nc.vector.tensor_copy(zrr[:, 4:8], zrr[:, 0:4])
            zrb = cpool.tile([128, 1], F32, tag="zrb")
            nc.sync.dma_start(out=zrb[:], in_=zrr[:])
            w = cpool.tile([128, 256], F32, tag="t3", name="w")
            nc.vector.tensor_scalar(w[:], wt[:], zrb[:], None, Alu.mult)
            nc.sync.dma_start(out=out[:], in_=w[:])
    return nc


def shard_inputs(k, beta, gamma, keys):
    k = np.ascontiguousarray(k, dtype=np.float32)
    beta = np.ascontiguousarray(beta, dtype=np.float32).reshape(B)
    gamma = np.ascontiguousarray(gamma, dtype=np.float32).reshape(B)
    keys = np.asarray(keys, dtype=np.float32)
    in_maps = []
    for c in range(M):
        sl = slice(c * BPC, (c + 1) * BPC)
        kc = k[sl]                                            # [BPC, K]
        keysTc = np.ascontiguousarray(
            keys[sl].transpose(2, 0, 1)).reshape(KQ, 128, BPC, N)
        # kvTd[p, s, q, c] = kvec_s[128q+p] if c == s else 0
        kvTd_c = np.zeros((128, BPC, KQ, 32), np.float32)
        for s in range(BPC):
            kvTd_c[:, s, :, s] = kc[s].reshape(KQ, 128).T
        # onesT[p, s, c] = 1.0 if c == 16 + s else 0
        onesT_c = np.zeros((128, BPC, 32), np.float32)
        for s in range(BPC):
            onesT_c[:, s, 16 + s] = 1.0
        qn = np.maximum(np.linalg.norm(kc.astype(np.float64), axis=-1), EPS)
        lnsb_s = np.log(beta[sl].astype(np.float64) / qn).astype(np.float32)
        gam_s = gamma[sl]
        # partition p = 8*s + c  ->  per-partition sample index p // 8
        rep = np.repeat(np.arange(BPC), 8)
        in_maps.append({
            "keysT": keysTc,
            "kvTd": kvTd_c.reshape(128, BPC * KQ * 32),
            "onesT": onesT_c.reshape(128, BPC * 32),
            "lnsb": lnsb_s[rep].reshape(128, 1),
            "gam": gam_s[rep].reshape(128, 1).astype(np.float32),
            "ng30g": (-SHIFT * gam_s[rep]).reshape(128, 1).astype(np.float32),
        })
    return in_maps


_NC_CACHE = None


def kernel(k=None, beta=None, gamma=None, keys=None, candidates=None,
           **_ignored):
    assert int(candidates) == CAND, \
        f"kernel hardcoded for candidates=32, got {candidates}"
    assert keys.shape == (B, N, K), keys.shape
    global _NC_CACHE
    if _NC_CACHE is None:
        _NC_CACHE = build_nc()
    in_maps = shard_inputs(k, beta, gamma, keys)
    res = run_bass_kernel_spmd(_NC_CACHE, in_maps, list(range(M))).results
    return np.concatenate(
        [res[c]["out"].reshape(BPC, N) for c in range(M)], axis=0)
